# revision 32
# baseline (speedup 1.0000x reference)
"""Trainium2 Bass kernel for C = triu(triu(A) @ triu(B)), N=4096, fp32.

2D sharding over 8 cores — 4 row-groups x 2 col-groups, identical SPMD
program on every core (one NEFF); only the per-core packed input DATA
differs. Host-side triu masking makes tiles of A with k < m and tiles of B
with k > c exactly zero, so the shared program may run matmuls over a
superset k-range; zero tiles contribute nothing.

Sharding: core j -> (i = j % 4, h = j // 4).
  Rows:    core owns 128-row tiles m = 4t + i, t = 0..7        (cyclic by 4)
  Columns: core owns 128-col tiles c = 2w + h, w = 0..15       (cyclic by 2)
Column tiles are grouped into 4 supers u = 0..3; super u covers the core's
own tiles {8u+h, 8u+2+h, 8u+4+h, 8u+6+h} packed into a 512-wide psum.
Output block (m=4t+i, super u) accumulates k in [4t, 8u+7]: 113,664 matmul
rows/core (vs 128k for 1D row sharding). Per-core HBM: A 4.72 MB +
B 8.91 MB + C(bf16 out) 2.62 MB = 16.3 MB.

Schedule — keep the PE streaming back-to-back (~0.42 ns/row) and the DMA
engines delivering exactly in consumption order:
- 18 dummy matmuls on a memset tile ride out the PE p-state ramp (0.65 ->
  2.4 GHz) during the initial DMA wait.
- ALL loads (8 A k-quads + every B chunk) issue on the single Sync-queue
  DGE ring, interleaved at their first-consumption points: a FIFO ring is
  a precise bandwidth allocator, so A never steals from the B stream.
- Supers run [1, 0, 3, 2]. u=1 starts with its diagonal k-range (needs only
  0.85 MB before the first real matmul). Tiny u=0 next gives the ring slack
  to build lead. u=3 runs high-k full-width chunks first (6 slots per B
  byte), its diagonal mid, and the cheap low-k chunks last, when the ring
  has caught up. u=2 (diag first, then descending k) closes with a minimal
  drain tail.
- Descending k staggers slot completion (slot t last touches k=4t), so
  PSUM->SBUF bf16 cast-copies (Vector/Scalar alternating) and C stores
  (GpSimd queue, 2-slot pairs = 2 KB DMA lines) fire mid-stream instead of
  bunching at super boundaries; the final two slots store individually so
  the drain tail is one copy + one 128x512 store.
"""

import sys

for _p in ("/opt/trn_rl_repo", "/root/.axon_site/_ro/trn_rl_repo"):
    if _p not in sys.path:
        sys.path.insert(0, _p)

import numpy as np

N = 4096
P = 128
NCORES = 8
NKT = 32
U_ORDER = [1, 0, 3, 2]
NT_DIAG = [4, 4, 3, 3, 2, 2, 1, 1]  # col-tiles touched at k = 8u+d
W_DIAG = [128 * n for n in NT_DIAG]
DCOL = [0, 512, 1024, 1408, 1792, 2048, 2304, 2432]
DIAG_COLS = 2560
N_WARM = 18  # dummy matmuls to ramp the PE p-state

# A: k-major (k, t) tile list, t <= k//4, loaded in 8 k-quad chunks
A_PAIRS = [(k, t) for k in range(NKT) for t in range(k // 4 + 1)]
A_IDX = {kt: i for i, kt in enumerate(A_PAIRS)}
NA = len(A_PAIRS)  # 144
A_QUAD_CNT = [sum(1 for k, _ in A_PAIRS if k // 4 == g) for g in range(8)]
A_QUAD_OFF = [sum(A_QUAD_CNT[:g]) for g in range(8)]


def _diag_halves(u):
    """Diagonal k-range of super u split into two chunks of 4 k's."""
    a = [(8 * u + d, W_DIAG[d], DCOL[d]) for d in range(4)]
    b = [(8 * u + d, W_DIAG[d], DCOL[d] - DCOL[4]) for d in range(4, 8)]
    return a, b


def _chunk_lists(u):
    """B chunks per super, in load/use (= execution) order. Each chunk is a
    list of (k, width, offset_in_chunk)."""
    da, db = _diag_halves(u)
    if u == 1:  # diag-first (flat DMA-demand profile), then full desc
        da1, da2 = da[:2], [(k, w, o - 1024) for k, w, o in da[2:]]
        return [
            da1,
            da2,
            db,
            [(k, 512, 512 * n) for n, k in enumerate([7, 6, 5, 4])],
            [(k, 512, 512 * n) for n, k in enumerate([3, 2, 1, 0])],
        ]
    if u == 0:
        return [[(d, W_DIAG[d], DCOL[d]) for d in range(8)]]
    # u=3: full-width descending first (A/B cheap, high slot count), diag last
    # u=2: diag first, then full-width descending (staggered stores, small tail)
    full = []
    for c0 in range(8 * u - 4, -1, -4):
        ch = list(range(c0 + 3, c0 - 1, -1))
        full.append([(k, 512, 512 * n) for n, k in enumerate(ch)])
    if u == 3:
        # high-k desc (max slots/byte), diag, then the cheap low-k chunks
        # last — by then the load ring has fully caught up
        return full[:4] + [da, db] + full[4:]
    return [da, db] + full


B_CHUNKS = {u: _chunk_lists(u) for u in range(4)}
B_CHUNK_COLS = {
    u: [sum(w for _, w, _ in ch) for ch in B_CHUNKS[u]] for u in range(4)
}
SEG_COLS = {u: sum(B_CHUNK_COLS[u]) for u in range(4)}
B_OFF = {}
_off = 0
for _u in U_ORDER:
    B_OFF[_u] = _off
    _off += SEG_COLS[_u]
BCOLS = _off  # 34816

C_OFF = {}
_off = 0
for _u in U_ORDER:
    C_OFF[_u] = _off
    _off += (2 * _u + 2) * 512
CCOLS = _off  # 10240


def _k_exec(u):
    """Execution order of k within super u (concatenated chunk k's)."""
    return [k for ch in B_CHUNKS[u] for k, _, _ in ch]


_cache = {}


def _build():
    import concourse.bacc as bacc
    import concourse.mybir as mybir
    import concourse.tile as tile

    BF = mybir.dt.bfloat16
    F32 = mybir.dt.float32

    nc = bacc.Bacc(None, target_bir_lowering=False)
    AT = nc.dram_tensor("AT", [P, NA, P], BF, kind="ExternalInput")
    Bm = nc.dram_tensor("B", [P, BCOLS], BF, kind="ExternalInput")
    Cm = nc.dram_tensor("C", [P, CCOLS], BF, kind="ExternalOutput")

    with tile.TileContext(nc) as tc:
        with (
            tc.tile_pool(name="a", bufs=1) as apool,
            tc.tile_pool(name="b", bufs=14) as bpool,
            tc.tile_pool(name="bd", bufs=3) as dpool,
            tc.tile_pool(name="o", bufs=5) as opool,
            tc.tile_pool(name="w", bufs=1) as wpool,
            tc.tile_pool(name="ps", bufs=8, space="PSUM") as pspool,
        ):
            # PE warm-up: memset a tile, then dummy matmuls into a psum that
            # the pool recycles later. Rides out the p-state ramp during the
            # initial DMA wait.
            warm = wpool.tile([P, 640], BF, tag="w", name="warm")
            nc.gpsimd.memset(warm[:], 0)
            ps_w = pspool.tile([P, 512], F32, tag="ps", name="ps_w")
            for i in range(N_WARM):
                nc.tensor.matmul(
                    ps_w[:],
                    warm[:, :128],
                    warm[:, 128:640],
                    start=(i == 0),
                    stop=(i == N_WARM - 1),
                )

            # A resident in 8 k-quad tiles; the loads are interleaved into
            # the single Sync-queue ring at their first-consumption points so
            # the DMA engines deliver A and B in exactly the order the PE
            # needs them (FIFO ring = precise bandwidth allocation).
            a_tiles = [
                apool.tile([P, A_QUAD_CNT[g], P], BF, tag=f"a{g}", name="ag")
                for g in range(8)
            ]

            def a_load(g):
                nc.sync.dma_start(
                    a_tiles[g][:],
                    AT[:, A_QUAD_OFF[g] : A_QUAD_OFF[g] + A_QUAD_CNT[g], :],
                )

            def a_ap(k, t):
                g = k // 4
                return a_tiles[g][:, A_IDX[(k, t)] - A_QUAD_OFF[g], :]

            # a-quad to load just before (super, chunk_index):
            # u1 diag uses q2, q3 then desc q1, q0; u3 desc needs q5 then q4,
            # its diag q6, q7.
            a_before = {
                (1, 0): 2,
                (1, 2): 3,
                (1, 3): 1,
                (1, 4): 0,
                (3, 0): 5,
                (3, 1): 4,
                (3, 4): 6,
                (3, 5): 7,
            }

            for u in U_ORDER:
                nslots = 2 * u + 2
                kmax = 8 * u + 7
                k_exec = _k_exec(u)
                # first/last executed position per slot
                first_pos, last_pos = {}, {}
                for pos, k in enumerate(k_exec):
                    for t in range(min(k // 4, 2 * u + 1) + 1):
                        first_pos.setdefault(t, pos)
                        last_pos[t] = pos

                psums = [
                    pspool.tile([P, 512], F32, tag="ps", name="ps")
                    for _ in range(nslots)
                ]

                # issue all B chunk loads for this super (prefetch via pool)
                rhs_of = {}
                coff = B_OFF[u]
                for ci, ch in enumerate(B_CHUNKS[u]):
                    if (u, ci) in a_before:
                        a_load(a_before[(u, ci)])
                    ncols = B_CHUNK_COLS[u][ci]
                    if ncols == DIAG_COLS:
                        bt = dpool.tile([P, DIAG_COLS], BF, tag="bd", name="bd")
                    else:
                        bt = bpool.tile([P, ncols], BF, tag="b", name="bt")
                    nc.sync.dma_start(bt[:], Bm[:, coff : coff + ncols])
                    for k, w, o in ch:
                        rhs_of[k] = (bt, o, w)
                    coff += ncols

                pair_tiles = {}
                pair_left = {}
                for pos, k in enumerate(k_exec):
                    bt, o, w = rhs_of[k]
                    rhs = bt[:, o : o + w]
                    for t in range(min(k // 4, 2 * u + 1) + 1):
                        nc.tensor.matmul(
                            psums[t][:, 512 - w : 512],
                            a_ap(k, t),
                            rhs,
                            start=(pos == first_pos[t]),
                            stop=(pos == last_pos[t]),
                        )
                        if pos == last_pos[t]:
                            if u == U_ORDER[-1] and t <= 1:
                                # tail-critical slots of the final super:
                                # individual copy+store, no pair-wait
                                ot1 = opool.tile(
                                    [P, 512], BF, tag=f"o1{t}", name="ot1"
                                )
                                if t == 0:
                                    nc.vector.tensor_copy(ot1[:], psums[0][:])
                                else:
                                    nc.scalar.copy(ot1[:], psums[1][:])
                                nc.gpsimd.dma_start(
                                    Cm[
                                        :,
                                        C_OFF[u] + 512 * t : C_OFF[u] + 512 * (t + 1),
                                    ],
                                    ot1[:],
                                )
                                continue
                            # slot complete: cast-copy into its pair staging
                            # tile; store the pair (2 KB DMA lines) when both
                            # halves are down.
                            p = t // 2
                            if p not in pair_tiles:
                                pair_tiles[p] = opool.tile(
                                    [P, 1024], BF, tag="o", name="ot"
                                )
                                pair_left[p] = 2
                            ot = pair_tiles[p]
                            half = ot[:, 512 * (t % 2) : 512 * (t % 2) + 512]
                            if t % 2 == 0:
                                nc.vector.tensor_copy(half, psums[t][:])
                            else:
                                nc.scalar.copy(half, psums[t][:])
                            pair_left[p] -= 1
                            if pair_left[p] == 0:
                                dst = Cm[
                                    :,
                                    C_OFF[u] + 1024 * p : C_OFF[u] + 1024 * (p + 1),
                                ]
                                nc.gpsimd.dma_start(dst, ot[:])
    nc.compile()
    return nc


def _get_nc():
    if "nc" not in _cache:
        _cache["nc"] = _build()
    return _cache["nc"]


def _make_in_maps(A, B):
    import ml_dtypes

    bf = np.dtype(ml_dtypes.bfloat16)
    A = np.asarray(A, dtype=np.float32)
    B = np.asarray(B, dtype=np.float32)
    Au = np.triu(A)
    Bu = np.triu(B)

    Au_t = Au.reshape(NKT, P, NKT, P)
    Bu_t = Bu.reshape(NKT, P, NKT, P)

    # A packs per row-offset i: tile (k, t) = Au[m=4t+i, k]^T -> [kr, mr]
    a_packs = []
    ks = np.array([k for k, _ in A_PAIRS])
    ts = np.array([t for _, t in A_PAIRS])
    for i in range(4):
        g = Au_t[4 * ts + i, :, ks, :]  # [NA, mr, kr]
        ATd = np.ascontiguousarray(g.transpose(2, 0, 1)).astype(bf)
        a_packs.append(ATd)

    # B packs per col-parity h, in chunk order
    b_packs = []
    for h in range(2):
        kl, cl = [], []
        for u in U_ORDER:
            for ch in B_CHUNKS[u]:
                for k, w, _ in ch:
                    nt = w // 128
                    for j in range(4 - nt, 4):
                        kl.append(k)
                        cl.append(8 * u + 2 * j + h)
        kl = np.array(kl)
        cl = np.array(cl)
        g = Bu_t[kl, :, cl, :]  # [ntiles, kr, cr]
        Bd = np.ascontiguousarray(
            g.transpose(1, 0, 2).reshape(P, len(kl) * P)
        ).astype(bf)
        assert Bd.shape[1] == BCOLS
        b_packs.append(Bd)

    return [{"AT": a_packs[j % 4], "B": b_packs[j // 4]} for j in range(NCORES)]


def kernel(A, B):
    from concourse.bass_utils import run_bass_kernel_spmd

    in_maps = _make_in_maps(A, B)
    nc = _get_nc()
    res = run_bass_kernel_spmd(nc, in_maps, core_ids=list(range(NCORES)))

    C = np.zeros((N, N), dtype=np.float32)
    for j in range(NCORES):
        i, h = j % 4, j // 4
        Cj = np.asarray(res.results[j]["C"], dtype=np.float32)
        for u in range(4):
            for t in range(2 * u + 2):
                m = 4 * t + i
                for jj in range(4):
                    c = 8 * u + 2 * jj + h
                    if c >= m:
                        C[P * m : P * (m + 1), P * c : P * (c + 1)] = Cj[
                            :,
                            C_OFF[u] + 512 * t + 128 * jj : C_OFF[u]
                            + 512 * t
                            + 128 * (jj + 1),
                        ]
    return C


# revision 33
# speedup vs baseline: 1.0313x; 1.0313x over previous
"""Trainium2 Bass kernel for C = triu(triu(A) @ triu(B)), N=4096, fp32.

2D sharding over 8 cores — 4 row-groups x 2 col-groups, identical SPMD
program on every core (one NEFF); only the per-core packed input DATA
differs. Host-side triu masking makes tiles of A with k < m and tiles of B
with k > c exactly zero, so the shared program may run matmuls over a
superset k-range; zero tiles contribute nothing.

Sharding: core j -> (i = j % 4, h = j // 4).
  Rows:    core owns 128-row tiles m = 4t + i, t = 0..7        (cyclic by 4)
  Columns: core owns 128-col tiles c = 2w + h, w = 0..15       (cyclic by 2)
Column tiles are grouped into 4 supers u = 0..3; super u covers the core's
own tiles {8u+h, 8u+2+h, 8u+4+h, 8u+6+h} packed into a 512-wide psum.
Output block (m=4t+i, super u) accumulates k in [4t, 8u+7]: 113,664 matmul
rows/core (vs 128k for 1D row sharding). Per-core HBM: A 4.72 MB +
B 8.91 MB + C(bf16 out) 2.62 MB = 16.3 MB.

Schedule — keep the PE streaming back-to-back (~0.42 ns/row) and the DMA
engines delivering exactly in consumption order:
- 18 dummy matmuls on a memset tile ride out the PE p-state ramp (0.65 ->
  2.4 GHz) during the initial DMA wait.
- ALL loads (8 A k-quads + every B chunk) issue on the single Sync-queue
  DGE ring, interleaved at their first-consumption points: a FIFO ring is
  a precise bandwidth allocator, so A never steals from the B stream.
- Supers run [1, 0, 3, 2]. u=1 starts with its diagonal k-range (needs only
  0.85 MB before the first real matmul). Tiny u=0 next gives the ring slack
  to build lead. u=3 runs high-k full-width chunks first (6 slots per B
  byte), its diagonal mid, and the cheap low-k chunks last, when the ring
  has caught up. u=2 (diag first, then descending k) closes with a minimal
  drain tail.
- Descending k staggers slot completion (slot t last touches k=4t), so
  PSUM->SBUF bf16 cast-copies (Vector/Scalar alternating) and C stores
  (GpSimd queue, 2-slot pairs = 2 KB DMA lines) fire mid-stream instead of
  bunching at super boundaries; the final two slots store individually so
  the drain tail is one copy + one 128x512 store.
"""

import sys

for _p in ("/opt/trn_rl_repo", "/root/.axon_site/_ro/trn_rl_repo"):
    if _p not in sys.path:
        sys.path.insert(0, _p)

import numpy as np

N = 4096
P = 128
NCORES = 8
NKT = 32
U_ORDER = [1, 0, 3, 2]
NT_DIAG = [4, 4, 3, 3, 2, 2, 1, 1]  # col-tiles touched at k = 8u+d
W_DIAG = [128 * n for n in NT_DIAG]
DCOL = [0, 512, 1024, 1408, 1792, 2048, 2304, 2432]
DIAG_COLS = 2560
N_WARM = 18  # dummy matmuls to ramp the PE p-state

# A: k-major (k, t) tile list, t <= k//4, loaded in 8 k-quad chunks
A_PAIRS = [(k, t) for k in range(NKT) for t in range(k // 4 + 1)]
A_IDX = {kt: i for i, kt in enumerate(A_PAIRS)}
NA = len(A_PAIRS)  # 144
A_QUAD_CNT = [sum(1 for k, _ in A_PAIRS if k // 4 == g) for g in range(8)]
A_QUAD_OFF = [sum(A_QUAD_CNT[:g]) for g in range(8)]


def _diag_halves(u):
    """Diagonal k-range of super u split into two chunks of 4 k's."""
    a = [(8 * u + d, W_DIAG[d], DCOL[d]) for d in range(4)]
    b = [(8 * u + d, W_DIAG[d], DCOL[d] - DCOL[4]) for d in range(4, 8)]
    return a, b


def _chunk_lists(u):
    """B chunks per super, in load/use (= execution) order. Each chunk is a
    list of (k, width, offset_in_chunk)."""
    da, db = _diag_halves(u)
    if u == 1:  # diag-first (flat DMA-demand profile), then full desc
        da1, da2 = da[:2], [(k, w, o - 1024) for k, w, o in da[2:]]
        return [
            da1,
            da2,
            db,
            [(k, 512, 512 * n) for n, k in enumerate([7, 6, 5, 4])],
            [(k, 512, 512 * n) for n, k in enumerate([3, 2, 1, 0])],
        ]
    if u == 0:
        return [[(d, W_DIAG[d], DCOL[d]) for d in range(8)]]
    # u=3: full-width descending first (A/B cheap, high slot count), diag last
    # u=2: diag first, then full-width descending (staggered stores, small tail)
    full = []
    for c0 in range(8 * u - 4, -1, -4):
        ch = list(range(c0 + 3, c0 - 1, -1))
        full.append([(k, 512, 512 * n) for n, k in enumerate(ch)])
    if u == 3:
        # high-k desc (max slots/byte), diag, then the cheap low-k chunks
        # last — by then the load ring has fully caught up
        return full[:4] + [da, db] + full[4:]
    return [da, db] + full


B_CHUNKS = {u: _chunk_lists(u) for u in range(4)}
B_CHUNK_COLS = {
    u: [sum(w for _, w, _ in ch) for ch in B_CHUNKS[u]] for u in range(4)
}
SEG_COLS = {u: sum(B_CHUNK_COLS[u]) for u in range(4)}
B_OFF = {}
_off = 0
for _u in U_ORDER:
    B_OFF[_u] = _off
    _off += SEG_COLS[_u]
BCOLS = _off  # 34816

C_OFF = {}
_off = 0
for _u in U_ORDER:
    C_OFF[_u] = _off
    _off += (2 * _u + 2) * 512
CCOLS = _off  # 10240


def _k_exec(u):
    """Execution order of k within super u (concatenated chunk k's)."""
    return [k for ch in B_CHUNKS[u] for k, _, _ in ch]


_cache = {}


def _build():
    import concourse.bacc as bacc
    import concourse.mybir as mybir
    import concourse.tile as tile

    BF = mybir.dt.bfloat16
    F32 = mybir.dt.float32

    nc = bacc.Bacc(None, target_bir_lowering=False)
    AT = nc.dram_tensor("AT", [P, NA, P], BF, kind="ExternalInput")
    Bm = nc.dram_tensor("B", [P, BCOLS], BF, kind="ExternalInput")
    Cm = nc.dram_tensor("C", [P, CCOLS], BF, kind="ExternalOutput")

    with tile.TileContext(nc) as tc:
        with (
            tc.tile_pool(name="a", bufs=1) as apool,
            tc.tile_pool(name="b", bufs=10) as bpool,
            tc.tile_pool(name="bd", bufs=3) as dpool,
            tc.tile_pool(name="o", bufs=5) as opool,
            tc.tile_pool(name="w", bufs=1) as wpool,
            tc.tile_pool(name="ps", bufs=8, space="PSUM") as pspool,
        ):
            # PE warm-up: memset a tile, then dummy matmuls into a psum that
            # the pool recycles later. Rides out the p-state ramp during the
            # initial DMA wait.
            warm = wpool.tile([P, 640], BF, tag="w", name="warm")
            nc.gpsimd.memset(warm[:], 0)
            ps_w = pspool.tile([P, 512], F32, tag="ps", name="ps_w")
            for i in range(N_WARM):
                nc.tensor.matmul(
                    ps_w[:],
                    warm[:, :128],
                    warm[:, 128:640],
                    start=(i == 0),
                    stop=(i == N_WARM - 1),
                )

            # A resident in 8 k-quad tiles; the loads are interleaved into
            # the single Sync-queue ring at their first-consumption points so
            # the DMA engines deliver A and B in exactly the order the PE
            # needs them (FIFO ring = precise bandwidth allocation).
            a_tiles = [
                apool.tile([P, A_QUAD_CNT[g], P], BF, tag=f"a{g}", name="ag")
                for g in range(8)
            ]

            def a_load(g):
                nc.sync.dma_start(
                    a_tiles[g][:],
                    AT[:, A_QUAD_OFF[g] : A_QUAD_OFF[g] + A_QUAD_CNT[g], :],
                )

            def a_ap(k, t):
                g = k // 4
                return a_tiles[g][:, A_IDX[(k, t)] - A_QUAD_OFF[g], :]

            # a-quad to load just before (super, chunk_index):
            # u1 diag uses q2, q3 then desc q1, q0; u3 desc needs q5 then q4,
            # its diag q6, q7.
            a_before = {
                (1, 0): 2,
                (1, 2): 3,
                (1, 3): 1,
                (1, 4): 0,
                (3, 0): 5,
                (3, 1): 4,
                (3, 4): 6,
                (3, 5): 7,
            }

            for u in U_ORDER:
                nslots = 2 * u + 2
                kmax = 8 * u + 7
                k_exec = _k_exec(u)
                # first/last executed position per slot
                first_pos, last_pos = {}, {}
                for pos, k in enumerate(k_exec):
                    for t in range(min(k // 4, 2 * u + 1) + 1):
                        first_pos.setdefault(t, pos)
                        last_pos[t] = pos

                psums = [
                    pspool.tile([P, 512], F32, tag="ps", name="ps")
                    for _ in range(nslots)
                ]

                # issue all B chunk loads for this super (prefetch via pool)
                rhs_of = {}
                coff = B_OFF[u]
                for ci, ch in enumerate(B_CHUNKS[u]):
                    if (u, ci) in a_before:
                        a_load(a_before[(u, ci)])
                    ncols = B_CHUNK_COLS[u][ci]
                    if ncols == DIAG_COLS:
                        bt = dpool.tile([P, DIAG_COLS], BF, tag="bd", name="bd")
                    else:
                        bt = bpool.tile([P, ncols], BF, tag="b", name="bt")
                    nc.sync.dma_start(bt[:], Bm[:, coff : coff + ncols])
                    for k, w, o in ch:
                        rhs_of[k] = (bt, o, w)
                    coff += ncols

                pair_tiles = {}
                pair_left = {}
                for pos, k in enumerate(k_exec):
                    bt, o, w = rhs_of[k]
                    rhs = bt[:, o : o + w]
                    for t in range(min(k // 4, 2 * u + 1) + 1):
                        nc.tensor.matmul(
                            psums[t][:, 512 - w : 512],
                            a_ap(k, t),
                            rhs,
                            start=(pos == first_pos[t]),
                            stop=(pos == last_pos[t]),
                        )
                        if pos == last_pos[t]:
                            if u == U_ORDER[-1] and t <= 1:
                                # tail-critical slots of the final super:
                                # individual copy+store, no pair-wait
                                ot1 = opool.tile(
                                    [P, 512], BF, tag=f"o1{t}", name="ot1"
                                )
                                if t == 0:
                                    nc.vector.tensor_copy(ot1[:], psums[0][:])
                                else:
                                    nc.scalar.copy(ot1[:], psums[1][:])
                                nc.gpsimd.dma_start(
                                    Cm[
                                        :,
                                        C_OFF[u] + 512 * t : C_OFF[u] + 512 * (t + 1),
                                    ],
                                    ot1[:],
                                )
                                continue
                            # slot complete: cast-copy into its pair staging
                            # tile; store the pair (2 KB DMA lines) when both
                            # halves are down.
                            p = t // 2
                            if p not in pair_tiles:
                                pair_tiles[p] = opool.tile(
                                    [P, 1024], BF, tag="o", name="ot"
                                )
                                pair_left[p] = 2
                            ot = pair_tiles[p]
                            half = ot[:, 512 * (t % 2) : 512 * (t % 2) + 512]
                            if t % 2 == 0:
                                nc.vector.tensor_copy(half, psums[t][:])
                            else:
                                nc.scalar.copy(half, psums[t][:])
                            pair_left[p] -= 1
                            if pair_left[p] == 0:
                                dst = Cm[
                                    :,
                                    C_OFF[u] + 1024 * p : C_OFF[u] + 1024 * (p + 1),
                                ]
                                nc.gpsimd.dma_start(dst, ot[:])
    nc.compile()
    return nc


def _get_nc():
    if "nc" not in _cache:
        _cache["nc"] = _build()
    return _cache["nc"]


def _make_in_maps(A, B):
    import ml_dtypes

    bf = np.dtype(ml_dtypes.bfloat16)
    A = np.asarray(A, dtype=np.float32)
    B = np.asarray(B, dtype=np.float32)
    Au = np.triu(A)
    Bu = np.triu(B)

    Au_t = Au.reshape(NKT, P, NKT, P)
    Bu_t = Bu.reshape(NKT, P, NKT, P)

    # A packs per row-offset i: tile (k, t) = Au[m=4t+i, k]^T -> [kr, mr]
    a_packs = []
    ks = np.array([k for k, _ in A_PAIRS])
    ts = np.array([t for _, t in A_PAIRS])
    for i in range(4):
        g = Au_t[4 * ts + i, :, ks, :]  # [NA, mr, kr]
        ATd = np.ascontiguousarray(g.transpose(2, 0, 1)).astype(bf)
        a_packs.append(ATd)

    # B packs per col-parity h, in chunk order
    b_packs = []
    for h in range(2):
        kl, cl = [], []
        for u in U_ORDER:
            for ch in B_CHUNKS[u]:
                for k, w, _ in ch:
                    nt = w // 128
                    for j in range(4 - nt, 4):
                        kl.append(k)
                        cl.append(8 * u + 2 * j + h)
        kl = np.array(kl)
        cl = np.array(cl)
        g = Bu_t[kl, :, cl, :]  # [ntiles, kr, cr]
        Bd = np.ascontiguousarray(
            g.transpose(1, 0, 2).reshape(P, len(kl) * P)
        ).astype(bf)
        assert Bd.shape[1] == BCOLS
        b_packs.append(Bd)

    return [{"AT": a_packs[j % 4], "B": b_packs[j // 4]} for j in range(NCORES)]


def kernel(A, B):
    from concourse.bass_utils import run_bass_kernel_spmd

    in_maps = _make_in_maps(A, B)
    nc = _get_nc()
    res = run_bass_kernel_spmd(nc, in_maps, core_ids=list(range(NCORES)))

    C = np.zeros((N, N), dtype=np.float32)
    for j in range(NCORES):
        i, h = j % 4, j // 4
        Cj = np.asarray(res.results[j]["C"], dtype=np.float32)
        for u in range(4):
            for t in range(2 * u + 2):
                m = 4 * t + i
                for jj in range(4):
                    c = 8 * u + 2 * jj + h
                    if c >= m:
                        C[P * m : P * (m + 1), P * c : P * (c + 1)] = Cj[
                            :,
                            C_OFF[u] + 512 * t + 128 * jj : C_OFF[u]
                            + 512 * t
                            + 128 * (jj + 1),
                        ]
    return C


# revision 35
# speedup vs baseline: 1.0479x; 1.0161x over previous
"""Trainium2 Bass kernel for C = triu(triu(A) @ triu(B)), N=4096, fp32.

2D sharding over 8 cores — 4 row-groups x 2 col-groups, identical SPMD
program on every core (one NEFF); only the per-core packed input DATA
differs. Host-side triu masking makes tiles of A with k < m and tiles of B
with k > c exactly zero, so the shared program may run matmuls over a
superset k-range; zero tiles contribute nothing.

Sharding: core j -> (i = j % 4, h = j // 4).
  Rows:    core owns 128-row tiles m = 4t + i, t = 0..7        (cyclic by 4)
  Columns: core owns 128-col tiles c = 2w + h, w = 0..15       (cyclic by 2)
Column tiles are grouped into 4 supers u = 0..3; super u covers the core's
own tiles {8u+h, 8u+2+h, 8u+4+h, 8u+6+h} packed into a 512-wide psum.
Output block (m=4t+i, super u) accumulates k in [4t, 8u+7]: 113,664 matmul
rows/core (vs 128k for 1D row sharding). Per-core HBM: A 4.72 MB +
B 8.91 MB + C(bf16 out) 2.62 MB = 16.3 MB.

Schedule — keep the PE streaming back-to-back (~0.42 ns/row) and the DMA
engines delivering exactly in consumption order:
- 18 dummy matmuls on a memset tile ride out the PE p-state ramp (0.65 ->
  2.4 GHz) during the initial DMA wait.
- ALL loads (8 A k-quads + every B chunk) issue on the single Sync-queue
  DGE ring, interleaved at their first-consumption points: a FIFO ring is
  a precise bandwidth allocator, so A never steals from the B stream.
- Supers run [1, 0, 3, 2]. u=1 starts with its diagonal k-range (needs only
  0.85 MB before the first real matmul). Tiny u=0 next gives the ring slack
  to build lead. u=3 runs high-k full-width chunks first (6 slots per B
  byte), its diagonal mid, and the cheap low-k chunks last, when the ring
  has caught up. u=2 (diag first, then descending k) closes with a minimal
  drain tail.
- Descending k staggers slot completion (slot t last touches k=4t), so
  PSUM->SBUF bf16 cast-copies (Vector/Scalar alternating) and C stores
  (GpSimd queue, 2-slot pairs = 2 KB DMA lines) fire mid-stream instead of
  bunching at super boundaries; the final two slots store individually so
  the drain tail is one copy + one 128x512 store.
"""

import sys

for _p in ("/opt/trn_rl_repo", "/root/.axon_site/_ro/trn_rl_repo"):
    if _p not in sys.path:
        sys.path.insert(0, _p)

import numpy as np

N = 4096
P = 128
NCORES = 8
NKT = 32
U_ORDER = [1, 0, 3, 2]
NT_DIAG = [4, 4, 3, 3, 2, 2, 1, 1]  # col-tiles touched at k = 8u+d
W_DIAG = [128 * n for n in NT_DIAG]
DCOL = [0, 512, 1024, 1408, 1792, 2048, 2304, 2432]
DIAG_COLS = 2560
N_WARM = 18  # dummy matmuls to ramp the PE p-state

# A: k-major (k, t) tile list, t <= k//4, loaded in 8 k-quad chunks
A_PAIRS = [(k, t) for k in range(NKT) for t in range(k // 4 + 1)]
A_IDX = {kt: i for i, kt in enumerate(A_PAIRS)}
NA = len(A_PAIRS)  # 144
A_QUAD_CNT = [sum(1 for k, _ in A_PAIRS if k // 4 == g) for g in range(8)]
A_QUAD_OFF = [sum(A_QUAD_CNT[:g]) for g in range(8)]


def _diag_halves(u):
    """Diagonal k-range of super u split into two chunks of 4 k's."""
    a = [(8 * u + d, W_DIAG[d], DCOL[d]) for d in range(4)]
    b = [(8 * u + d, W_DIAG[d], DCOL[d] - DCOL[4]) for d in range(4, 8)]
    return a, b


def _chunk_lists(u):
    """B chunks per super, in load/use (= execution) order. Each chunk is a
    list of (k, width, offset_in_chunk)."""
    da, db = _diag_halves(u)
    if u == 1:  # diag-first (flat DMA-demand profile), then full desc
        da1, da2 = da[:2], [(k, w, o - 1024) for k, w, o in da[2:]]
        return [
            da1,
            da2,
            db,
            [(k, 512, 512 * n) for n, k in enumerate([7, 6, 5, 4])],
            [(k, 512, 512 * n) for n, k in enumerate([3, 2, 1, 0])],
        ]
    if u == 0:
        return [[(d, W_DIAG[d], DCOL[d]) for d in range(8)]]
    # u=3: full-width descending first (A/B cheap, high slot count), diag last
    # u=2: diag first, then full-width descending (staggered stores, small tail)
    full = []
    for c0 in range(8 * u - 4, -1, -4):
        ch = list(range(c0 + 3, c0 - 1, -1))
        full.append([(k, 512, 512 * n) for n, k in enumerate(ch)])
    if u == 3:
        # high-k desc (max slots/byte), diag, then the cheap low-k chunks
        # last — by then the load ring has fully caught up
        return full[:4] + [da, db] + full[4:]
    return [da, db] + full


B_CHUNKS = {u: _chunk_lists(u) for u in range(4)}
B_CHUNK_COLS = {
    u: [sum(w for _, w, _ in ch) for ch in B_CHUNKS[u]] for u in range(4)
}
SEG_COLS = {u: sum(B_CHUNK_COLS[u]) for u in range(4)}
B_OFF = {}
_off = 0
for _u in U_ORDER:
    B_OFF[_u] = _off
    _off += SEG_COLS[_u]
BCOLS = _off  # 34816

C_OFF = {}
_off = 0
for _u in U_ORDER:
    C_OFF[_u] = _off
    _off += (2 * _u + 2) * 512
CCOLS = _off  # 10240


def _k_exec(u):
    """Execution order of k within super u (concatenated chunk k's)."""
    return [k for ch in B_CHUNKS[u] for k, _, _ in ch]


_cache = {}


def _build():
    import concourse.bacc as bacc
    import concourse.mybir as mybir
    import concourse.tile as tile

    BF = mybir.dt.bfloat16
    F32 = mybir.dt.float32

    nc = bacc.Bacc(None, target_bir_lowering=False)
    AT = nc.dram_tensor("AT", [P, NA, P], BF, kind="ExternalInput")
    Bm = nc.dram_tensor("B", [P, BCOLS], BF, kind="ExternalInput")
    Cm = nc.dram_tensor("C", [P, CCOLS], BF, kind="ExternalOutput")

    with tile.TileContext(nc) as tc:
        with (
            tc.tile_pool(name="a", bufs=1) as apool,
            tc.tile_pool(name="b", bufs=10) as bpool,
            tc.tile_pool(name="bd", bufs=3) as dpool,
            tc.tile_pool(name="o", bufs=5) as opool,
            tc.tile_pool(name="w", bufs=1) as wpool,
            tc.tile_pool(name="ps", bufs=8, space="PSUM") as pspool,
        ):
            # PE warm-up: memset a tile, then dummy matmuls into a psum that
            # the pool recycles later. Rides out the p-state ramp during the
            # initial DMA wait.
            warm = wpool.tile([P, 640], BF, tag="w", name="warm")
            nc.gpsimd.memset(warm[:], 0)
            ps_w = pspool.tile([P, 512], F32, tag="ps", name="ps_w")
            for i in range(N_WARM):
                nc.tensor.matmul(
                    ps_w[:],
                    warm[:, :128],
                    warm[:, 128:640],
                    start=(i == 0),
                    stop=(i == N_WARM - 1),
                )

            # A resident in 8 k-quad tiles; the loads are interleaved into
            # the single Sync-queue ring at their first-consumption points so
            # the DMA engines deliver A and B in exactly the order the PE
            # needs them (FIFO ring = precise bandwidth allocation).
            a_tiles = [
                apool.tile([P, A_QUAD_CNT[g], P], BF, tag=f"a{g}", name="ag")
                for g in range(8)
            ]

            def a_load(g):
                nc.sync.dma_start(
                    a_tiles[g][:],
                    AT[:, A_QUAD_OFF[g] : A_QUAD_OFF[g] + A_QUAD_CNT[g], :],
                )

            def a_ap(k, t):
                g = k // 4
                return a_tiles[g][:, A_IDX[(k, t)] - A_QUAD_OFF[g], :]

            # a-quad to load just before (super, chunk_index):
            # u1 diag uses q2, q3 then desc q1, q0; u3 desc needs q5 then q4,
            # its diag q6, q7.
            a_before = {
                (1, 0): 2,
                (1, 2): 3,
                (1, 3): 1,
                (1, 4): 0,
                (3, 0): 5,
                (3, 1): 4,
                (3, 4): 6,
                (3, 5): 7,
            }

            for u in U_ORDER:
                nslots = 2 * u + 2
                kmax = 8 * u + 7
                k_exec = _k_exec(u)
                # first/last executed position per slot
                first_pos, last_pos = {}, {}
                for pos, k in enumerate(k_exec):
                    for t in range(min(k // 4, 2 * u + 1) + 1):
                        first_pos.setdefault(t, pos)
                        last_pos[t] = pos

                psums = [
                    pspool.tile([P, 512], F32, tag="ps", name="ps")
                    for _ in range(nslots)
                ]

                # issue all B chunk loads for this super (prefetch via pool)
                rhs_of = {}
                coff = B_OFF[u]
                for ci, ch in enumerate(B_CHUNKS[u]):
                    if (u, ci) in a_before:
                        a_load(a_before[(u, ci)])
                    ncols = B_CHUNK_COLS[u][ci]
                    if ncols == DIAG_COLS:
                        bt = dpool.tile([P, DIAG_COLS], BF, tag="bd", name="bd")
                    else:
                        bt = bpool.tile([P, ncols], BF, tag="b", name="bt")
                    nc.sync.dma_start(bt[:], Bm[:, coff : coff + ncols])
                    for k, w, o in ch:
                        rhs_of[k] = (bt, o, w)
                    coff += ncols

                pair_tiles = {}
                pair_left = {}
                for pos, k in enumerate(k_exec):
                    bt, o, w = rhs_of[k]
                    rhs = bt[:, o : o + w]
                    for t in range(min(k // 4, 2 * u + 1) + 1):
                        nc.tensor.matmul(
                            psums[t][:, 512 - w : 512],
                            a_ap(k, t),
                            rhs,
                            start=(pos == first_pos[t]),
                            stop=(pos == last_pos[t]),
                        )
                        if pos == last_pos[t]:
                            if u == U_ORDER[-1] and t <= 1:
                                # tail-critical slots of the final super:
                                # individual copy+store, no pair-wait
                                ot1 = opool.tile(
                                    [P, 512], BF, tag=f"o1{t}", name="ot1"
                                )
                                if t == 0:
                                    nc.vector.tensor_copy(ot1[:], psums[0][:])
                                else:
                                    nc.scalar.copy(ot1[:], psums[1][:])
                                nc.scalar.dma_start(
                                    Cm[
                                        :,
                                        C_OFF[u] + 512 * t : C_OFF[u] + 512 * (t + 1),
                                    ],
                                    ot1[:],
                                )
                                continue
                            # slot complete: cast-copy into its pair staging
                            # tile; store the pair (2 KB DMA lines) when both
                            # halves are down.
                            p = t // 2
                            if p not in pair_tiles:
                                pair_tiles[p] = opool.tile(
                                    [P, 1024], BF, tag="o", name="ot"
                                )
                                pair_left[p] = 2
                            ot = pair_tiles[p]
                            half = ot[:, 512 * (t % 2) : 512 * (t % 2) + 512]
                            if t % 2 == 0:
                                nc.vector.tensor_copy(half, psums[t][:])
                            else:
                                nc.scalar.copy(half, psums[t][:])
                            pair_left[p] -= 1
                            if pair_left[p] == 0:
                                dst = Cm[
                                    :,
                                    C_OFF[u] + 1024 * p : C_OFF[u] + 1024 * (p + 1),
                                ]
                                nc.scalar.dma_start(dst, ot[:])
    nc.compile()
    return nc


def _get_nc():
    if "nc" not in _cache:
        _cache["nc"] = _build()
    return _cache["nc"]


def _make_in_maps(A, B):
    import ml_dtypes

    bf = np.dtype(ml_dtypes.bfloat16)
    A = np.asarray(A, dtype=np.float32)
    B = np.asarray(B, dtype=np.float32)
    Au = np.triu(A)
    Bu = np.triu(B)

    Au_t = Au.reshape(NKT, P, NKT, P)
    Bu_t = Bu.reshape(NKT, P, NKT, P)

    # A packs per row-offset i: tile (k, t) = Au[m=4t+i, k]^T -> [kr, mr]
    a_packs = []
    ks = np.array([k for k, _ in A_PAIRS])
    ts = np.array([t for _, t in A_PAIRS])
    for i in range(4):
        g = Au_t[4 * ts + i, :, ks, :]  # [NA, mr, kr]
        ATd = np.ascontiguousarray(g.transpose(2, 0, 1)).astype(bf)
        a_packs.append(ATd)

    # B packs per col-parity h, in chunk order
    b_packs = []
    for h in range(2):
        kl, cl = [], []
        for u in U_ORDER:
            for ch in B_CHUNKS[u]:
                for k, w, _ in ch:
                    nt = w // 128
                    for j in range(4 - nt, 4):
                        kl.append(k)
                        cl.append(8 * u + 2 * j + h)
        kl = np.array(kl)
        cl = np.array(cl)
        g = Bu_t[kl, :, cl, :]  # [ntiles, kr, cr]
        Bd = np.ascontiguousarray(
            g.transpose(1, 0, 2).reshape(P, len(kl) * P)
        ).astype(bf)
        assert Bd.shape[1] == BCOLS
        b_packs.append(Bd)

    return [{"AT": a_packs[j % 4], "B": b_packs[j // 4]} for j in range(NCORES)]


def kernel(A, B):
    from concourse.bass_utils import run_bass_kernel_spmd

    in_maps = _make_in_maps(A, B)
    nc = _get_nc()
    res = run_bass_kernel_spmd(nc, in_maps, core_ids=list(range(NCORES)))

    C = np.zeros((N, N), dtype=np.float32)
    for j in range(NCORES):
        i, h = j % 4, j // 4
        Cj = np.asarray(res.results[j]["C"], dtype=np.float32)
        for u in range(4):
            for t in range(2 * u + 2):
                m = 4 * t + i
                for jj in range(4):
                    c = 8 * u + 2 * jj + h
                    if c >= m:
                        C[P * m : P * (m + 1), P * c : P * (c + 1)] = Cj[
                            :,
                            C_OFF[u] + 512 * t + 128 * jj : C_OFF[u]
                            + 512 * t
                            + 128 * (jj + 1),
                        ]
    return C


# revision 36
# speedup vs baseline: 1.0730x; 1.0239x over previous
"""Trainium2 Bass kernel for C = triu(triu(A) @ triu(B)), N=4096, fp32.

2D sharding over 8 cores — 4 row-groups x 2 col-groups, identical SPMD
program on every core (one NEFF); only the per-core packed input DATA
differs. Host-side triu masking makes tiles of A with k < m and tiles of B
with k > c exactly zero, so the shared program may run matmuls over a
superset k-range; zero tiles contribute nothing.

Sharding: core j -> (i = j % 4, h = j // 4).
  Rows:    core owns 128-row tiles m = 4t + i, t = 0..7        (cyclic by 4)
  Columns: core owns 128-col tiles c = 2w + h, w = 0..15       (cyclic by 2)
Column tiles are grouped into 4 supers u = 0..3; super u covers the core's
own tiles {8u+h, 8u+2+h, 8u+4+h, 8u+6+h} packed into a 512-wide psum.
Output block (m=4t+i, super u) accumulates k in [4t, 8u+7]: 113,664 matmul
rows/core (vs 128k for 1D row sharding). Per-core HBM: A 4.72 MB +
B 8.91 MB + C(bf16 out) 2.62 MB = 16.3 MB.

Schedule — keep the PE streaming back-to-back (~0.42 ns/row) and the DMA
engines delivering exactly in consumption order:
- 18 dummy matmuls on a memset tile ride out the PE p-state ramp (0.65 ->
  2.4 GHz) during the initial DMA wait.
- ALL loads (8 A k-quads + every B chunk) issue on the single Sync-queue
  DGE ring, interleaved at their first-consumption points: a FIFO ring is
  a precise bandwidth allocator, so A never steals from the B stream.
- Supers run [1, 0, 3, 2]. u=1 starts with its diagonal k-range (needs only
  0.85 MB before the first real matmul). Tiny u=0 next gives the ring slack
  to build lead. u=3 runs high-k full-width chunks first (6 slots per B
  byte), its diagonal mid, and the cheap low-k chunks last, when the ring
  has caught up. u=2 (diag first, then descending k) closes with a minimal
  drain tail.
- Descending k staggers slot completion (slot t last touches k=4t), so
  PSUM->SBUF bf16 cast-copies (Vector/Scalar alternating) and C stores
  (GpSimd queue, 2-slot pairs = 2 KB DMA lines) fire mid-stream instead of
  bunching at super boundaries; the final two slots store individually so
  the drain tail is one copy + one 128x512 store.
"""

import sys

for _p in ("/opt/trn_rl_repo", "/root/.axon_site/_ro/trn_rl_repo"):
    if _p not in sys.path:
        sys.path.insert(0, _p)

import numpy as np

N = 4096
P = 128
NCORES = 8
NKT = 32
U_ORDER = [1, 0, 3, 2]
NT_DIAG = [4, 4, 3, 3, 2, 2, 1, 1]  # col-tiles touched at k = 8u+d
W_DIAG = [128 * n for n in NT_DIAG]
DCOL = [0, 512, 1024, 1408, 1792, 2048, 2304, 2432]
DIAG_COLS = 2560
N_WARM = 18  # dummy matmuls to ramp the PE p-state

# A: k-major (k, t) tile list, t <= k//4, loaded in 8 k-quad chunks
A_PAIRS = [(k, t) for k in range(NKT) for t in range(k // 4 + 1)]
A_IDX = {kt: i for i, kt in enumerate(A_PAIRS)}
NA = len(A_PAIRS)  # 144
A_QUAD_CNT = [sum(1 for k, _ in A_PAIRS if k // 4 == g) for g in range(8)]
A_QUAD_OFF = [sum(A_QUAD_CNT[:g]) for g in range(8)]


def _diag_halves(u):
    """Diagonal k-range of super u split into two chunks of 4 k's."""
    a = [(8 * u + d, W_DIAG[d], DCOL[d]) for d in range(4)]
    b = [(8 * u + d, W_DIAG[d], DCOL[d] - DCOL[4]) for d in range(4, 8)]
    return a, b


def _chunk_lists(u):
    """B chunks per super, in load/use (= execution) order. Each chunk is a
    list of (k, width, offset_in_chunk)."""
    da, db = _diag_halves(u)
    if u == 1:  # diag-first (flat DMA-demand profile), then full desc
        da1, da2 = da[:2], [(k, w, o - 1024) for k, w, o in da[2:]]
        return [
            da1,
            da2,
            db,
            [(k, 512, 512 * n) for n, k in enumerate([7, 6, 5, 4])],
            [(k, 512, 512 * n) for n, k in enumerate([3, 2, 1, 0])],
        ]
    if u == 0:
        return [[(d, W_DIAG[d], DCOL[d]) for d in range(8)]]
    # u=3: full-width descending first (A/B cheap, high slot count), diag last
    # u=2: diag first, then full-width descending (staggered stores, small tail)
    full = []
    for c0 in range(8 * u - 4, -1, -4):
        ch = list(range(c0 + 3, c0 - 1, -1))
        full.append([(k, 512, 512 * n) for n, k in enumerate(ch)])
    if u == 3:
        # high-k desc (max slots/byte), diag, then the cheap low-k chunks
        # last — by then the load ring has fully caught up
        return full[:4] + [da, db] + full[4:]
    return [da, db] + full


B_CHUNKS = {u: _chunk_lists(u) for u in range(4)}
B_CHUNK_COLS = {
    u: [sum(w for _, w, _ in ch) for ch in B_CHUNKS[u]] for u in range(4)
}
SEG_COLS = {u: sum(B_CHUNK_COLS[u]) for u in range(4)}
B_OFF = {}
_off = 0
for _u in U_ORDER:
    B_OFF[_u] = _off
    _off += SEG_COLS[_u]
BCOLS = _off  # 34816

C_OFF = {}
_off = 0
for _u in U_ORDER:
    C_OFF[_u] = _off
    _off += (2 * _u + 2) * 512
CCOLS = _off  # 10240


def _k_exec(u):
    """Execution order of k within super u (concatenated chunk k's)."""
    return [k for ch in B_CHUNKS[u] for k, _, _ in ch]


_cache = {}


def _build():
    import concourse.bacc as bacc
    import concourse.mybir as mybir
    import concourse.tile as tile

    BF = mybir.dt.bfloat16
    F32 = mybir.dt.float32

    nc = bacc.Bacc(None, target_bir_lowering=False)
    AT = nc.dram_tensor("AT", [P, NA, P], BF, kind="ExternalInput")
    Bm = nc.dram_tensor("B", [P, BCOLS], BF, kind="ExternalInput")
    Cm = nc.dram_tensor("C", [P, CCOLS], BF, kind="ExternalOutput")

    with tile.TileContext(nc) as tc:
        with (
            tc.tile_pool(name="a", bufs=1) as apool,
            tc.tile_pool(name="b", bufs=10) as bpool,
            tc.tile_pool(name="bd", bufs=3) as dpool,
            tc.tile_pool(name="o", bufs=5) as opool,
            tc.tile_pool(name="w", bufs=1) as wpool,
            tc.tile_pool(name="ps", bufs=8, space="PSUM") as pspool,
        ):
            # PE warm-up: memset a tile, then dummy matmuls into a psum that
            # the pool recycles later. Rides out the p-state ramp during the
            # initial DMA wait.
            warm = wpool.tile([P, 640], BF, tag="w", name="warm")
            nc.vector.memset(warm[:], 0)
            ps_w = pspool.tile([P, 512], F32, tag="ps", name="ps_w")
            for i in range(N_WARM):
                nc.tensor.matmul(
                    ps_w[:],
                    warm[:, :128],
                    warm[:, 128:640],
                    start=(i == 0),
                    stop=(i == N_WARM - 1),
                )

            # A resident in 8 k-quad tiles; the loads are interleaved into
            # the single Sync-queue ring at their first-consumption points so
            # the DMA engines deliver A and B in exactly the order the PE
            # needs them (FIFO ring = precise bandwidth allocation).
            a_tiles = [
                apool.tile([P, A_QUAD_CNT[g], P], BF, tag=f"a{g}", name="ag")
                for g in range(8)
            ]

            def a_load(g):
                nc.sync.dma_start(
                    a_tiles[g][:],
                    AT[:, A_QUAD_OFF[g] : A_QUAD_OFF[g] + A_QUAD_CNT[g], :],
                )

            def a_ap(k, t):
                g = k // 4
                return a_tiles[g][:, A_IDX[(k, t)] - A_QUAD_OFF[g], :]

            # a-quad to load just before (super, chunk_index):
            # u1 diag uses q2, q3 then desc q1, q0; u3 desc needs q5 then q4,
            # its diag q6, q7.
            a_before = {
                (1, 0): 2,
                (1, 2): 3,
                (1, 3): 1,
                (1, 4): 0,
                (3, 0): 5,
                (3, 1): 4,
                (3, 4): 6,
                (3, 5): 7,
            }

            for u in U_ORDER:
                nslots = 2 * u + 2
                kmax = 8 * u + 7
                k_exec = _k_exec(u)
                # first/last executed position per slot
                first_pos, last_pos = {}, {}
                for pos, k in enumerate(k_exec):
                    for t in range(min(k // 4, 2 * u + 1) + 1):
                        first_pos.setdefault(t, pos)
                        last_pos[t] = pos

                psums = [
                    pspool.tile([P, 512], F32, tag="ps", name="ps")
                    for _ in range(nslots)
                ]

                # issue all B chunk loads for this super (prefetch via pool)
                rhs_of = {}
                coff = B_OFF[u]
                for ci, ch in enumerate(B_CHUNKS[u]):
                    if (u, ci) in a_before:
                        a_load(a_before[(u, ci)])
                    ncols = B_CHUNK_COLS[u][ci]
                    if ncols == DIAG_COLS:
                        bt = dpool.tile([P, DIAG_COLS], BF, tag="bd", name="bd")
                    else:
                        bt = bpool.tile([P, ncols], BF, tag="b", name="bt")
                    nc.sync.dma_start(bt[:], Bm[:, coff : coff + ncols])
                    for k, w, o in ch:
                        rhs_of[k] = (bt, o, w)
                    coff += ncols

                pair_tiles = {}
                pair_left = {}
                for pos, k in enumerate(k_exec):
                    bt, o, w = rhs_of[k]
                    rhs = bt[:, o : o + w]
                    for t in range(min(k // 4, 2 * u + 1) + 1):
                        nc.tensor.matmul(
                            psums[t][:, 512 - w : 512],
                            a_ap(k, t),
                            rhs,
                            start=(pos == first_pos[t]),
                            stop=(pos == last_pos[t]),
                        )
                        if pos == last_pos[t]:
                            if u == U_ORDER[-1] and t <= 1:
                                # tail-critical slots of the final super:
                                # individual copy+store, no pair-wait
                                ot1 = opool.tile(
                                    [P, 512], BF, tag=f"o1{t}", name="ot1"
                                )
                                if t == 0:
                                    nc.vector.tensor_copy(ot1[:], psums[0][:])
                                else:
                                    nc.scalar.copy(ot1[:], psums[1][:])
                                nc.scalar.dma_start(
                                    Cm[
                                        :,
                                        C_OFF[u] + 512 * t : C_OFF[u] + 512 * (t + 1),
                                    ],
                                    ot1[:],
                                )
                                continue
                            # slot complete: cast-copy into its pair staging
                            # tile; store the pair (2 KB DMA lines) when both
                            # halves are down.
                            p = t // 2
                            if p not in pair_tiles:
                                pair_tiles[p] = opool.tile(
                                    [P, 1024], BF, tag="o", name="ot"
                                )
                                pair_left[p] = 2
                            ot = pair_tiles[p]
                            half = ot[:, 512 * (t % 2) : 512 * (t % 2) + 512]
                            if t % 2 == 0:
                                nc.vector.tensor_copy(half, psums[t][:])
                            else:
                                nc.scalar.copy(half, psums[t][:])
                            pair_left[p] -= 1
                            if pair_left[p] == 0:
                                dst = Cm[
                                    :,
                                    C_OFF[u] + 1024 * p : C_OFF[u] + 1024 * (p + 1),
                                ]
                                nc.scalar.dma_start(dst, ot[:])
    nc.compile()
    return nc


def _get_nc():
    if "nc" not in _cache:
        _cache["nc"] = _build()
    return _cache["nc"]


def _make_in_maps(A, B):
    import ml_dtypes

    bf = np.dtype(ml_dtypes.bfloat16)
    A = np.asarray(A, dtype=np.float32)
    B = np.asarray(B, dtype=np.float32)
    Au = np.triu(A)
    Bu = np.triu(B)

    Au_t = Au.reshape(NKT, P, NKT, P)
    Bu_t = Bu.reshape(NKT, P, NKT, P)

    # A packs per row-offset i: tile (k, t) = Au[m=4t+i, k]^T -> [kr, mr]
    a_packs = []
    ks = np.array([k for k, _ in A_PAIRS])
    ts = np.array([t for _, t in A_PAIRS])
    for i in range(4):
        g = Au_t[4 * ts + i, :, ks, :]  # [NA, mr, kr]
        ATd = np.ascontiguousarray(g.transpose(2, 0, 1)).astype(bf)
        a_packs.append(ATd)

    # B packs per col-parity h, in chunk order
    b_packs = []
    for h in range(2):
        kl, cl = [], []
        for u in U_ORDER:
            for ch in B_CHUNKS[u]:
                for k, w, _ in ch:
                    nt = w // 128
                    for j in range(4 - nt, 4):
                        kl.append(k)
                        cl.append(8 * u + 2 * j + h)
        kl = np.array(kl)
        cl = np.array(cl)
        g = Bu_t[kl, :, cl, :]  # [ntiles, kr, cr]
        Bd = np.ascontiguousarray(
            g.transpose(1, 0, 2).reshape(P, len(kl) * P)
        ).astype(bf)
        assert Bd.shape[1] == BCOLS
        b_packs.append(Bd)

    return [{"AT": a_packs[j % 4], "B": b_packs[j // 4]} for j in range(NCORES)]


def kernel(A, B):
    from concourse.bass_utils import run_bass_kernel_spmd

    in_maps = _make_in_maps(A, B)
    nc = _get_nc()
    res = run_bass_kernel_spmd(nc, in_maps, core_ids=list(range(NCORES)))

    C = np.zeros((N, N), dtype=np.float32)
    for j in range(NCORES):
        i, h = j % 4, j // 4
        Cj = np.asarray(res.results[j]["C"], dtype=np.float32)
        for u in range(4):
            for t in range(2 * u + 2):
                m = 4 * t + i
                for jj in range(4):
                    c = 8 * u + 2 * jj + h
                    if c >= m:
                        C[P * m : P * (m + 1), P * c : P * (c + 1)] = Cj[
                            :,
                            C_OFF[u] + 512 * t + 128 * jj : C_OFF[u]
                            + 512 * t
                            + 128 * (jj + 1),
                        ]
    return C
